# revision 12
# baseline (speedup 1.0000x reference)
"""CLIP text block (pre-LN causal attention + tanh-GELU MLP) on 8 trn2 cores.

Sharding: sequence-parallel. Core c handles query rows [512*(c%4), 512*(c%4+1))
of batch c//4. Each core computes K/V for its own rows, AllGathers K/V within
its 4-core batch group, then runs causal attention + MLP for its rows.

On-chip layout is feature-major ("transposed": [feature partitions, tokens]) so
every matmul consumes weights in natural [in_dim, out_dim] layout as lhsT.
Causality is enforced by multiplying exp(scores) tiles with 0/1 masks built
from an iota and a per-core threshold input. All matmul operands use float32r
(~2e-4 operand rounding, 4x faster than fp32 on the PE).
"""
import os
import sys

_TRN_REPO = "/opt/trn_rl_repo"
if _TRN_REPO not in sys.path:
    sys.path.insert(0, _TRN_REPO)

import numpy as np
import concourse.bass as bass
import concourse.mybir as mybir
import concourse.tile as tile
from concourse import bacc
from concourse.bass_utils import run_bass_kernel_spmd
from concourse.masks import make_identity

f32 = mybir.dt.float32
f32r = mybir.dt.float32r
bf16 = mybir.dt.bfloat16
AF = mybir.ActivationFunctionType
ALU = mybir.AluOpType

B, T, D, H, DH, FF = 2, 2048, 768, 12, 64, 3072
NCORES = 8
CH = 512            # query rows per core
P = 128
KD = D // P         # 6 feature tiles
NPAIR = H // 2      # 6 head pairs
NJT = T // P        # 16 key tiles
NIT = CH // P       # 4 token tiles per chunk
NSL = 4             # MLP ff slices of 768
FSL = FF // NSL     # 768
KFS = FSL // P      # 6 ff tiles per slice
EPS = 1e-5
ISCALE = 1.0 / 8.0  # 1/sqrt(DH)
KVW = P * KD * CH + NIT * P * H * (DH + 1)  # per-core gather payload (f32 words)
KT_W = P * KD * CH  # KT_own words


def _build():
    nc = bacc.Bacc("TRN2", target_bir_lowering=False, debug=False,
                   num_devices=NCORES)

    x_c = nc.dram_tensor("x_c", [CH, D], f32, kind="ExternalInput").ap()
    wq = nc.dram_tensor("wq", [D, D], f32r, kind="ExternalInput").ap()
    wk = nc.dram_tensor("wk", [D, D], f32r, kind="ExternalInput").ap()
    wv = nc.dram_tensor("wv", [D, D], f32r, kind="ExternalInput").ap()
    wo = nc.dram_tensor("wo", [D, D], f32r, kind="ExternalInput").ap()
    w1 = nc.dram_tensor("w1", [D, FF], f32r, kind="ExternalInput").ap()
    w2 = nc.dram_tensor("w2", [FF, D], f32r, kind="ExternalInput").ap()
    ln1_g = nc.dram_tensor("ln1_g", [D], f32, kind="ExternalInput").ap()
    ln1_b = nc.dram_tensor("ln1_b", [D], f32, kind="ExternalInput").ap()
    ln2_g = nc.dram_tensor("ln2_g", [D], f32, kind="ExternalInput").ap()
    ln2_b = nc.dram_tensor("ln2_b", [D], f32, kind="ExternalInput").ap()
    bq = nc.dram_tensor("bq", [D], f32, kind="ExternalInput").ap()
    bk = nc.dram_tensor("bk", [D], f32, kind="ExternalInput").ap()
    bv = nc.dram_tensor("bv", [D], f32r, kind="ExternalInput").ap()
    bo = nc.dram_tensor("bo", [D], f32, kind="ExternalInput").ap()
    b1 = nc.dram_tensor("b1", [FF], f32, kind="ExternalInput").ap()
    b2 = nc.dram_tensor("b2", [D], f32, kind="ExternalInput").ap()
    thr = nc.dram_tensor("thr", [P, NJT], f32, kind="ExternalInput").ap()
    y_c = nc.dram_tensor("y_c", [CH, D], f32, kind="ExternalOutput").ap()
    dbg = {}
    if os.environ.get("KDBG"):
        for nm, shp in [("xT", [P, KD, CH]), ("hT", [P, KD, CH]),
                        ("QT", [P, KD, CH]), ("KTown", [P, KD, CH]),
                        ("attnO", [DH, H, CH]), ("y1T", [P, KD, CH]),
                        ("h2T", [P, KD, CH]),
                        ("rstdd", [1, CH]), ("nmrd", [1, CH]),
                        ("KTgd", [P, KD, T]), ("Vgd", [P, NJT, H, DH + 1]),
                        ("oP0", [DH + 1, CH])]:
            dbg[nm] = nc.dram_tensor("dbg_" + nm, shp, f32,
                                     kind="ExternalOutput").ap()
        dbg["masksd"] = nc.dram_tensor("dbg_masksd", [P, NJT, CH], bf16,
                                       kind="ExternalOutput").ap()

    with tile.TileContext(nc) as tc:
        _body(nc, tc, x_c, wq, wk, wv, wo, w1, w2, ln1_g, ln1_b, ln2_g, ln2_b,
              bq, bk, bv, bo, b1, b2, thr, y_c, dbg)
    nc.compile()
    return nc


def _body(nc, tc, x_c, wq, wk, wv, wo, w1, w2, ln1_g, ln1_b, ln2_g, ln2_b,
          bq, bk, bv, bo, b1, b2, thr, y_c, dbg=None):
    def dump(nm, t):
        if dbg:
            nc.sync.dma_start(dbg[nm], t[:].bitcast(f32))
    with (
        tc.tile_pool(name="cst", bufs=1) as cst,
        tc.tile_pool(name="pers", bufs=1) as pers,
        tc.tile_pool(name="dram", bufs=1, space="DRAM") as dram,
    ):
        # ---- constants & params ----
        ident = cst.tile([P, P], f32)
        make_identity(nc, ident[:])
        iota_t = cst.tile([P, CH], f32)
        nc.gpsimd.iota(iota_t[:], pattern=[[1, CH]], base=0,
                       channel_multiplier=-1,
                       allow_small_or_imprecise_dtypes=True)
        ones_col = cst.tile([P, 1], f32)      # bitcast f32r when needed
        nc.vector.memset(ones_col[:], 1.0)
        ones_row = cst.tile([1, P], f32)
        nc.vector.memset(ones_row[:], 1.0)
        eps_t = cst.tile([P, 1], f32)
        nc.vector.memset(eps_t[:], EPS)
        ones65 = cst.tile([DH + 1, DH], f32)  # row 64 of ones, for denom bcast
        nc.vector.memset(ones65[DH:DH + 1, :], 1.0)

        def vec_pt(ap, n, name):  # [n*128] -> [128, n]
            t = cst.tile([P, n], f32, tag=name)
            nc.sync.dma_start(t[:], ap.rearrange("(t p) -> p t", p=P))
            return t

        ln1g_sb = vec_pt(ln1_g, KD, "ln1g")
        ln1b_sb = vec_pt(ln1_b, KD, "ln1b")
        ln2g_sb = vec_pt(ln2_g, KD, "ln2g")
        ln2b_sb = vec_pt(ln2_b, KD, "ln2b")
        bq_sb = vec_pt(bq, KD, "bqv")
        bk_sb = vec_pt(bk, KD, "bkv")
        bo_sb = vec_pt(bo, KD, "bov")
        b2_sb = vec_pt(b2, KD, "b2v")
        b1_sb = vec_pt(b1, FF // P, "b1v")
        thr_sb = cst.tile([P, NJT], f32)
        nc.sync.dma_start(thr_sb[:], thr)
        bv_row = cst.tile([1, D], f32r)
        nc.sync.dma_start(bv_row[:], bv[None, :])

        # ---- persistent activations ----
        xT = pers.tile([P, KD, CH], f32)        # x^T, feature-major
        QT = pers.tile([P, KD, CH], f32r)       # q^T (head pairs)
        attnO = pers.tile([DH, H, CH], f32r)    # softmax(QK)V / denom, ^T
        y1T = pers.tile([P, KD, CH], f32)       # x + attn out, feature-major

        kv_in = dram.tile([KVW], f32r)
        kv_out = dram.tile([4 * KVW], f32r)

        # ================= phase 1: LN1, QKV, gather =================
        with (
            tc.tile_pool(name="ph1", bufs=1) as ph1,
            tc.tile_pool(name="ph1s", bufs=2) as ph1s,
            tc.tile_pool(name="psA", bufs=2, space="PSUM") as psA,
            tc.tile_pool(name="psA1", bufs=1, space="PSUM") as psA1,
        ):
            wq_sb = ph1.tile([P, KD, D], f32r, tag="wq")
            nc.sync.dma_start(wq_sb[:], wq.rearrange("(k p) m -> p k m", p=P))
            wk_sb = ph1.tile([P, KD, D], f32r, tag="wk")
            nc.sync.dma_start(wk_sb[:], wk.rearrange("(k p) m -> p k m", p=P))
            wv_sb = ph1.tile([P, KD, D], f32r, tag="wv")
            nc.sync.dma_start(wv_sb[:], wv.rearrange("(k p) m -> p k m", p=P))

            # bv broadcast to all partitions: [128, 768]
            bvb_sb = ph1.tile([P, D], f32, tag="bvb")
            for g in range(2):
                bv_ps = psA.tile([P, 384], f32, tag="v")
                nc.tensor.matmul(bv_ps[:], ones_row[:].bitcast(f32r),
                                 bv_row[0:1, 384 * g:384 * (g + 1)],
                                 start=True, stop=True)
                nc.vector.tensor_copy(bvb_sb[:, 384 * g:384 * (g + 1)], bv_ps[:])

            # LN1 stats per token tile (natural layout), x transpose, h^T
            rstd_row = ph1.tile([1, CH], f32r, tag="rstdr")
            nmr_row = ph1.tile([1, CH], f32r, tag="nmrr")
            for it in range(NIT):
                xn = ph1s.tile([P, D], f32, tag="xn")
                nc.sync.dma_start(xn[:], x_c[P * it:P * (it + 1), :])
                ssum = ph1s.tile([P, 1], f32, tag="ssum")
                nc.vector.tensor_reduce(ssum[:], xn[:],
                                        axis=mybir.AxisListType.X, op=ALU.add)
                scr = ph1s.tile([P, D], f32, tag="scr")
                sqs = ph1s.tile([P, 1], f32, tag="sqs")
                nc.scalar.activation(scr[:], xn[:], AF.Square, accum_out=sqs[:])
                mu = ph1s.tile([P, 1], f32, tag="mu")
                nc.vector.tensor_scalar_mul(mu[:], ssum[:], 1.0 / D)
                e2 = ph1s.tile([P, 1], f32, tag="e2")
                nc.vector.tensor_scalar_mul(e2[:], sqs[:], 1.0 / D)
                musq = ph1s.tile([P, 1], f32, tag="musq")
                nc.vector.tensor_tensor(musq[:], mu[:], mu[:], ALU.mult)
                var = ph1s.tile([P, 1], f32, tag="var")
                nc.vector.tensor_tensor(var[:], e2[:], musq[:], ALU.subtract)
                std = ph1s.tile([P, 1], f32, tag="std")
                nc.scalar.activation(std[:], var[:], AF.Sqrt, bias=eps_t[:])
                rstd = ph1s.tile([P, 1], f32, tag="rstd")
                nc.vector.reciprocal(rstd[:], std[:])
                nmr = ph1s.tile([P, 1], f32, tag="nmr")
                nc.vector.tensor_tensor(nmr[:], mu[:], rstd[:], ALU.mult)
                nc.vector.tensor_scalar_mul(nmr[:], nmr[:], -1.0)

                # transpose the two stat columns to rows
                for src, dst in ((rstd, rstd_row), (nmr, nmr_row)):
                    r_ps = psA1.tile([1, P], f32, tag="r")
                    nc.tensor.transpose(r_ps[:], src[:], ident[:])
                    nc.vector.tensor_copy(dst[0:1, P * it:P * (it + 1)], r_ps[:])

                # transpose x tile into xT
                for k in range(KD):
                    t_ps = psA1.tile([P, P], f32, tag="t")
                    nc.tensor.transpose(t_ps[:], xn[:, P * k:P * (k + 1)],
                                        ident[:])
                    nc.vector.tensor_copy(xT[:, k, P * it:P * (it + 1)],
                                          t_ps[:])

            # broadcast rstd/nmr rows to 128 partitions
            bc_r = psA1.tile([P, CH], f32, tag="bcr")
            nc.tensor.matmul(bc_r[:], ones_row[:].bitcast(f32r), rstd_row[:],
                             start=True, stop=True)
            bc_n = psA1.tile([P, CH], f32, tag="bcn")
            nc.tensor.matmul(bc_n[:], ones_row[:].bitcast(f32r), nmr_row[:],
                             start=True, stop=True)

            hT = ph1.tile([P, KD, CH], f32r, tag="hT")
            for k in range(KD):
                tmp = ph1s.tile([P, CH], f32, tag="lnt")
                nc.vector.tensor_tensor(tmp[:], xT[:, k, :], bc_r[:], ALU.mult)
                nc.vector.tensor_tensor(tmp[:], tmp[:], bc_n[:], ALU.add)
                nc.scalar.activation(hT[:, k, :], tmp[:], AF.Identity,
                                     bias=ln1b_sb[:, k:k + 1],
                                     scale=ln1g_sb[:, k:k + 1])

            # Q^T and K^T per head pair: [128, 512] covers 2 heads
            KT_own = ph1.tile([P, KD, CH], f32r, tag="ktown")
            dump("rstdd", rstd_row)
            dump("nmrd", nmr_row)
            dump("xT", xT)
            dump("hT", hT)
            for t in range(NPAIR):
                q_ps = psA.tile([P, CH], f32, tag="qk")
                for k in range(KD):
                    nc.tensor.matmul(q_ps[:], wq_sb[:, k, P * t:P * (t + 1)],
                                     hT[:, k, :], start=(k == 0),
                                     stop=(k == KD - 1))
                nc.scalar.activation(QT[:, t, :], q_ps[:], AF.Identity,
                                     bias=bq_sb[:, t:t + 1])
                k_ps = psA.tile([P, CH], f32, tag="qk")
                for k in range(KD):
                    nc.tensor.matmul(k_ps[:], wk_sb[:, k, P * t:P * (t + 1)],
                                     hT[:, k, :], start=(k == 0),
                                     stop=(k == KD - 1))
                nc.scalar.activation(KT_own[:, t, :], k_ps[:], AF.Identity,
                                     bias=bk_sb[:, t:t + 1])
            nc.sync.dma_start(
                kv_in[0:KT_W].rearrange("(p x) -> p x", p=P),
                KT_own[:].rearrange("p a b -> p (a b)"))

            dump("QT", QT)
            dump("KTown", KT_own)
            # V natural layout per token tile, with ones column appended
            for it in range(NIT):
                v_own = ph1s.tile([P, H, DH + 1], f32r, tag="vown")
                for g in range(2):
                    v_ps = psA.tile([P, 384], f32, tag="v")
                    for k in range(KD):
                        nc.tensor.matmul(v_ps[:],
                                         hT[:, k, P * it:P * (it + 1)],
                                         wv_sb[:, k, 384 * g:384 * (g + 1)],
                                         start=(k == 0), stop=(k == KD - 1))
                    for hh in range(6):
                        h = 6 * g + hh
                        nc.vector.tensor_tensor(
                            v_own[:, h, 0:DH], v_ps[:, DH * hh:DH * (hh + 1)],
                            bvb_sb[:, DH * h:DH * (h + 1)], ALU.add)
                nc.vector.memset(v_own[:, :, DH:DH + 1].bitcast(f32), 1.0)
                # layout must match the gather-side read: [p, it, h*(DH+1)]
                nc.sync.dma_start(
                    kv_in[KT_W:KVW].rearrange("(p a x) -> p a x", p=P,
                                              a=NIT)[:, it, :],
                    v_own[:].rearrange("p a b -> p (a b)"))

        nc.gpsimd.collective_compute(
            "AllGather", ALU.bypass,
            replica_groups=[[0, 1, 2, 3], [4, 5, 6, 7]],
            ins=[kv_in[:].opt()], outs=[kv_out[:].opt()])

        # ================= phase 2: attention + o_proj =================
        with (
            tc.tile_pool(name="ph2", bufs=1) as ph2,
            tc.tile_pool(name="ph2s", bufs=1 if os.environ.get("KDBG") else 2) as ph2s,
            tc.tile_pool(name="psB", bufs=1, space="PSUM") as psB,
            tc.tile_pool(name="psBs", bufs=2, space="PSUM") as psBs,
        ):
            KTg = ph2.tile([P, KD, T], f32r, tag="ktg")
            Vg = ph2.tile([P, NJT, H, DH + 1], f32r, tag="vg")
            for g in range(4):
                nc.sync.dma_start(
                    KTg[:, :, CH * g:CH * (g + 1)],
                    kv_out[g * KVW:g * KVW + KT_W].rearrange(
                        "(p a i) -> p a i", p=P, a=KD))
                nc.sync.dma_start(
                    Vg[:, 4 * g:4 * (g + 1), :, :].rearrange(
                        "p a b c -> p (a b c)"),
                    kv_out[g * KVW + KT_W:(g + 1) * KVW].rearrange(
                        "(p x) -> p x", p=P))

            masks = ph2.tile([P, NJT, CH], bf16, tag="masks")
            for jt in range(NJT):
                nc.vector.tensor_scalar(masks[:, jt, :], iota_t[:],
                                        thr_sb[:, jt:jt + 1], None, ALU.is_ge)
            if dbg:
                nc.sync.dma_start(dbg["masksd"], masks[:])
                dump("KTgd", KTg)
                dump("Vgd", Vg)

            for h in range(H):
                t, u = h // 2, (h % 2) * DH
                o_ps = psBs.tile([DH + 1, CH], f32, tag="o")
                for jt in range(NJT):
                    s_ps = psBs.tile([P, CH], f32, tag="s")
                    nc.tensor.matmul(s_ps[:],
                                     KTg[u:u + DH, t, P * jt:P * (jt + 1)],
                                     QT[u:u + DH, t, :], start=True, stop=True)
                    e_sb = ph2s.tile([P, CH], f32, tag="exp")
                    nc.scalar.activation(e_sb[:], s_ps[:], AF.Exp,
                                         scale=ISCALE)
                    em_sb = ph2s.tile([P, CH], f32r, tag="expm")
                    nc.vector.tensor_tensor(em_sb[:], e_sb[:],
                                            masks[:, jt, :], ALU.mult)
                    nc.tensor.matmul(o_ps[:], Vg[:, jt, h, :], em_sb[:],
                                     start=(jt == 0), stop=(jt == NJT - 1))
                if dbg and h == 0:
                    o_sb_d = ph2.tile([DH + 1, CH], f32, tag="osbd")
                    nc.vector.tensor_copy(o_sb_d[:], o_ps[:])
                    dump("oP0", o_sb_d)
                rcp = ph2s.tile([DH + 1, CH], f32r, tag="rcp")
                with nc.allow_low_precision(reason="f32r softmax denom"):
                    nc.vector.reciprocal(rcp[DH:DH + 1, :], o_ps[DH:DH + 1, :])
                rb_ps = psB.tile([DH, CH], f32, tag="rb")
                nc.tensor.matmul(rb_ps[:], ones65[DH:DH + 1, :].bitcast(f32r),
                                 rcp[DH:DH + 1, :], start=True, stop=True)
                rb_sb = ph2s.tile([DH, CH], f32, tag="rbs")
                nc.vector.tensor_copy(rb_sb[:], rb_ps[:])
                nc.vector.tensor_tensor(attnO[:, h, :], o_ps[0:DH, :],
                                        rb_sb[:], ALU.mult)

            dump("attnO", attnO)
            # o_proj + residual -> y1T
            for m in range(KD):
                wo_sb = ph2s.tile([DH, H, P], f32r, tag="wo")
                nc.sync.dma_start(
                    wo_sb[:],
                    wo.rearrange("(h p) m -> p h m", p=DH)[:, :,
                                                           P * m:P * (m + 1)])
                o_mm = psBs.tile([P, CH], f32, tag="omm")
                for h in range(H):
                    nc.tensor.matmul(o_mm[:], wo_sb[:, h, :], attnO[:, h, :],
                                     start=(h == 0), stop=(h == H - 1))
                nc.vector.tensor_tensor(y1T[:, m, :], o_mm[:], xT[:, m, :],
                                        ALU.add)
                nc.scalar.activation(y1T[:, m, :], y1T[:, m, :], AF.Identity,
                                     bias=bo_sb[:, m:m + 1])

        # ================= phase 3: LN2 + MLP + output =================
        with (
            tc.tile_pool(name="ph3", bufs=1) as ph3,
            tc.tile_pool(name="ph3s", bufs=2) as ph3s,
        ):
            with tc.tile_pool(name="psL", bufs=1, space="PSUM") as psL:
                sum_ps = psL.tile([1, CH], f32, tag="sum")
                sq_ps = psL.tile([1, CH], f32, tag="sq")
                for k in range(KD):
                    nc.tensor.matmul(sum_ps[:], ones_col[:], y1T[:, k, :],
                                     start=(k == 0), stop=(k == KD - 1))
                for k in range(KD):
                    sq_sb = ph3s.tile([P, CH], f32r, tag="sqs")
                    nc.scalar.activation(sq_sb[:], y1T[:, k, :], AF.Square)
                    nc.tensor.matmul(sq_ps[:], ones_col[:].bitcast(f32r),
                                     sq_sb[:], start=(k == 0),
                                     stop=(k == KD - 1))
                mu2 = ph3s.tile([1, CH], f32, tag="mu2")
                nc.scalar.activation(mu2[:], sum_ps[:], AF.Copy, scale=1.0 / D)
                e22 = ph3s.tile([1, CH], f32, tag="e22")
                nc.scalar.activation(e22[:], sq_ps[:], AF.Copy, scale=1.0 / D)
                musq2 = ph3s.tile([1, CH], f32, tag="musq2")
                nc.vector.tensor_tensor(musq2[:], mu2[:], mu2[:], ALU.mult)
                var2 = ph3s.tile([1, CH], f32, tag="var2")
                nc.vector.tensor_tensor(var2[:], e22[:], musq2[:],
                                        ALU.subtract)
                std2 = ph3s.tile([1, CH], f32, tag="std2")
                nc.scalar.activation(std2[:], var2[:], AF.Sqrt, bias=eps_t[0:1, :])
                rstd2 = ph3s.tile([1, CH], f32r, tag="rstd2")
                with nc.allow_low_precision(reason="f32r ln2 rstd"):
                    nc.vector.reciprocal(rstd2[:], std2[:])
                nmr2 = ph3s.tile([1, CH], f32r, tag="nmr2")
                nc.vector.tensor_tensor(nmr2[:], mu2[:], rstd2[:], ALU.mult)
                nc.vector.tensor_scalar_mul(nmr2[:], nmr2[:], -1.0)
                bc_r2 = psL.tile([P, CH], f32, tag="bcr2")
                nc.tensor.matmul(bc_r2[:], ones_row[:].bitcast(f32r),
                                 rstd2[:], start=True, stop=True)
                bc_n2 = psL.tile([P, CH], f32, tag="bcn2")
                nc.tensor.matmul(bc_n2[:], ones_row[:].bitcast(f32r),
                                 nmr2[:], start=True, stop=True)
                h2T = ph3.tile([P, KD, CH], f32r, tag="h2T")
                for k in range(KD):
                    tmp = ph3s.tile([P, CH], f32, tag="lnt2")
                    nc.vector.tensor_tensor(tmp[:], y1T[:, k, :], bc_r2[:],
                                            ALU.mult)
                    nc.vector.tensor_tensor(tmp[:], tmp[:], bc_n2[:], ALU.add)
                    nc.scalar.activation(h2T[:, k, :], tmp[:], AF.Identity,
                                         bias=ln2b_sb[:, k:k + 1],
                                         scale=ln2g_sb[:, k:k + 1])

            dump("y1T", y1T)
            dump("h2T", h2T)
            yT = ph3.tile([P, KD, CH], f32, tag="yT")
            with (
                tc.tile_pool(name="psM", bufs=1, space="PSUM") as psM,
                tc.tile_pool(name="psZ", bufs=2, space="PSUM") as psZ,
            ):
                y2_ps = [psM.tile([P, CH], f32, tag=f"y2_{m}",
                                  name=f"y2_{m}")
                         for m in range(KD)]
                for s in range(NSL):
                    zs = ph3s.tile([P, KFS, CH], f32r, tag="zs")
                    for m in range(KFS):
                        z_ps = psZ.tile([P, CH], f32, tag="z")
                        w1t = ph3s.tile([P, KD, P], f32r, tag="w1t")
                        col = FSL * s + P * m
                        nc.sync.dma_start(
                            w1t[:],
                            w1.rearrange("(k p) f -> p k f",
                                         p=P)[:, :, col:col + P])
                        for k in range(KD):
                            nc.tensor.matmul(z_ps[:], w1t[:, k, :],
                                             h2T[:, k, :],
                                             start=(k == 0),
                                             stop=(k == KD - 1))
                        nc.scalar.activation(
                            zs[:, m, :], z_ps[:], AF.Gelu_apprx_tanh,
                            bias=b1_sb[:, KFS * s + m:KFS * s + m + 1])
                    for m2 in range(KD):
                        w2t = ph3s.tile([P, KFS, P], f32r, tag="w2t")
                        nc.sync.dma_start(
                            w2t[:],
                            w2[FSL * s:FSL * (s + 1),
                               P * m2:P * (m2 + 1)].rearrange(
                                   "(k p) d -> p k d", p=P))
                        for k in range(KFS):
                            nc.tensor.matmul(y2_ps[m2][:], w2t[:, k, :],
                                             zs[:, k, :],
                                             start=(s == 0 and k == 0),
                                             stop=(s == NSL - 1 and
                                                   k == KFS - 1))
                for m in range(KD):
                    nc.vector.tensor_tensor(yT[:, m, :], y2_ps[m][:],
                                            y1T[:, m, :], ALU.add)
                    nc.scalar.activation(yT[:, m, :], yT[:, m, :], AF.Identity,
                                         bias=b2_sb[:, m:m + 1])

            # transpose back to natural layout and store
            with tc.tile_pool(name="psO", bufs=2, space="PSUM") as psO:
                for it in range(NIT):
                    yn = ph3s.tile([P, D], f32, tag="yn")
                    for k in range(KD):
                        yt_ps = psO.tile([P, P], f32, tag="yt")
                        nc.tensor.transpose(yt_ps[:],
                                            yT[:, k, P * it:P * (it + 1)],
                                            ident[:])
                        nc.vector.tensor_copy(yn[:, P * k:P * (k + 1)],
                                              yt_ps[:])
                    nc.sync.dma_start(y_c[P * it:P * (it + 1), :], yn[:])


_NC = None


def _get_nc():
    global _NC
    if _NC is None:
        _NC = _build()
    return _NC


def make_in_maps(x, ln1_g, ln1_b, wq, bq, wk, bk, wv, bv, wo, bo,
                 ln2_g, ln2_b, w1, b1, w2, b2):
    c32 = lambda a: np.ascontiguousarray(np.asarray(a), dtype=np.float32)
    shared = dict(
        wq=c32(wq), wk=c32(wk), wv=c32(wv), wo=c32(wo), w1=c32(w1),
        w2=c32(w2), ln1_g=c32(ln1_g), ln1_b=c32(ln1_b), ln2_g=c32(ln2_g),
        ln2_b=c32(ln2_b), bq=c32(bq), bk=c32(bk), bv=c32(bv), bo=c32(bo),
        b1=c32(b1), b2=c32(b2))
    xf = c32(x)
    in_maps = []
    for c in range(NCORES):
        b, q = c // 4, c % 4
        thr_np = np.broadcast_to(
            (P * np.arange(NJT, dtype=np.float32) - CH * q)[None, :],
            (P, NJT)).copy()
        in_maps.append(dict(shared, x_c=xf[b, CH * q:CH * (q + 1)].copy(),
                            thr=thr_np))
    return in_maps


def kernel(**inputs):
    nc = _get_nc()
    in_maps = make_in_maps(**inputs)
    res = run_bass_kernel_spmd(nc, in_maps, core_ids=list(range(NCORES)))
    y = np.empty((B, T, D), np.float32)
    for c in range(NCORES):
        b, q = c // 4, c % 4
        y[b, CH * q:CH * (q + 1)] = res.results[c]["y_c"]
    return y
